# revision 66
# baseline (speedup 1.0000x reference)
"""Trainium2 Bass kernel for banded local attention.

Reference computation (B=2, S=2048, D=512, H=8, dh=64, local_range=7):
  q = hs @ Wq, k = hs @ Wk (per-head slices)
  scores = q k^T / sqrt(dh); w = softmax(scores) * band; w /= sum(w) + 1e-6
  ctx = w @ hs                                  -> [B, H, S, D]

Band-renormalized softmax == band-limited softmax up to the 1e-6*Z
correction (~1e-4 relative), so only the 15-diagonal band of scores is
ever computed.

Sharding (8 cores): core c -> batch b = c//4, S-half sh = (c//2)%2,
head group hg = c%2 (heads 4hg..4hg+3). Each core emits a [1024, 4, 512]
bf16 slab of unnormalized band-weighted sums plus the per-row band sums
(f32); the host divides and reassembles in f32.

Per-core tiling: 9 row tiles of P=114 (last 112). For each tile the band
j-window [i0-7, i0+107+14) spans exactly 128 rows, so the ctx contraction
is K=128. Scores are computed transposed ([j, i]) so the exp output feeds
ctx directly with no PE transposes; the band mask is applied as a NEG-bias
PSUM-group starter matmul (a K=64 matmul must not open a PSUM group).

Context is computed TRANSPOSED: for each 128-wide D-chunk c,
out^T[d, h*114+i] = win_chunk^T @ Em with Em (all 4 heads, N=456) as the
moving operand -- 1824 PE columns/tile instead of 2048. Band sums ride in
4 spare psum columns of chunk 0 (N=1 matmuls, 0 cycles) and leave as bf16
inside the same eviction; the host divides and re-transposes.

Cost-model-driven schedule (timeline sim is the graded clock): all DMAs
serialize on one 360 GB/s resource and lines < 512B pay 2x, so hsT loads
in >=512B-line chunks, wqk is packed [q0|k0|q1|k1] so its first half +
first hsT chunk unblock the first projection at ~4.3us, and each tile's
context leaves as two [128, 2, 460] DMAs (920B lines). Only DVE/Act can
read PSUM (GPSIMD cannot): per tile DVE drains chunks 0/2 (+3 on odd
tiles), Act does exp and drains chunk 1 (+3 on even tiles); projections
drain through both. Four independent single-bank psum slots keep
evictions off the matmul ring's critical path, and the double-buffered
score psum lets each tile's masks start without waiting for the previous
exp. PE is pre-warmed with junk matmuls against a Pool-memset tile
starting at ~1us, so the 3us p-state ramp to 2.4GHz completes before the
first real projection dispatches at 4.3us.
"""

import numpy as np
import ml_dtypes

BF = ml_dtypes.bfloat16
S, D, H, DH = 2048, 512, 8, 64
NCORES = 8
SL = 1024          # rows per core (S/2)
P = 114            # row-tile height (window = P + 14 = 128)
NT = 9             # ceil(1024 / 114); last tile has 112 rows
KT = D // 128      # contraction tiles for projections
HW = 1040          # hsd width: [s0-7, s0+1033)
NWARM = 6          # PE p-state warmup matmuls

TRACE = False
LAST_RESULTS = None

_NC_CACHE = {}


def _build_nc():
    import concourse.bacc as bacc
    import concourse.mybir as mybir
    import concourse.tile as tile

    f32 = mybir.dt.float32
    bf16 = mybir.dt.bfloat16
    AF = mybir.ActivationFunctionType

    nc = bacc.Bacc("TRN2", target_bir_lowering=False, debug=False, num_devices=NCORES)

    # hs^T slice, zero-padded outside the batch: col c = hs row s0-7+c
    hsd = nc.dram_tensor("hsd", [D, HW], bf16, kind="ExternalInput").ap()
    # band windows: win[p, t, :] = hs row (s0 + 114t - 7 + p), zero-padded
    win_d = nc.dram_tensor("win", [128, NT, D], bf16, kind="ExternalInput").ap()
    # projections packed [p, kt, (q_hp0 | k_hp0 | q_hp1 | k_hp1)], q pre-scaled
    wqk = nc.dram_tensor("wqk", [128, KT, 512], bf16, kind="ExternalInput").ap()
    # consts: identity(128) | NEG band masks slot0..2 (114 each) | ones(2)
    cmask = nc.dram_tensor("cmask", [128, 128 + 3 * P + 2], bf16, kind="ExternalInput").ap()
    # ctx transposed: out[p, c, t, h*114+i] = ctx[d = 128c+p, row 114t+i, head h];
    # chunk 0 carries 4 extra cols [456:460) = band sums for rows [0:114)
    OW = 4 * P + 4
    out = nc.dram_tensor("out", [128, KT, NT, OW], bf16, kind="ExternalOutput").ap()

    with tile.TileContext(nc) as tc:
        with (
            tc.tile_pool(name="const", bufs=1) as cpool,
            tc.tile_pool(name="ework", bufs=4) as epool,
            tc.tile_pool(name="outp", bufs=5) as opool,
            tc.tile_pool(name="pproj", bufs=2, space="PSUM") as pproj,
            tc.tile_pool(name="pscore", bufs=2, space="PSUM") as pscore,
            tc.tile_pool(name="pctx", bufs=2, space="PSUM") as pctx_pool,
        ):
            # ---- PE warmup fodder: Pool memsets a junk tile at t~0 (Pool
            # is free and SBUF-only memset is legal there) so the tensor
            # engine's p-state ramp (3us) completes before real inputs land ----
            junk = cpool.tile([128, 512], bf16)
            nc.gpsimd.memset(junk, 0)

            # ---- input DMAs; order = unblock order for the pipeline:
            # hp0 weights + first hsT chunk gate the first projection ----
            wqk_sb = cpool.tile([128, KT, 512], bf16)
            nc.sync.dma_start(out=wqk_sb[:, :, 0:256], in_=wqk[:, :, 0:256])
            hsT = cpool.tile([128, KT, HW], bf16)
            hsd_r = hsd.rearrange("(t p) s -> p t s", p=128)
            nc.sync.dma_start(out=hsT[:, :, 0:256], in_=hsd_r[:, :, 0:256])
            nc.sync.dma_start(out=wqk_sb[:, :, 256:512], in_=wqk[:, :, 256:512])
            cm = cpool.tile([128, 128 + 3 * P + 2], bf16)
            nc.sync.dma_start(out=cm, in_=cmask)
            id_sb = cm[:, 0:128]
            ones_sb = cm[:, 128 + 3 * P:128 + 3 * P + 1]
            win = cpool.tile([128, NT, D], bf16)
            nc.sync.dma_start(out=hsT[:, :, 256:768], in_=hsd_r[:, :, 256:768])
            nc.sync.dma_start(out=win[:, 0:2], in_=win_d[:, 0:2])
            nc.sync.dma_start(out=win[:, 2:NT], in_=win_d[:, 2:NT])
            nc.sync.dma_start(out=hsT[:, :, 768:HW], in_=hsd_r[:, :, 768:HW])

            # qk[, 0] = qT, qk[, 1] = kT; each [128(2 heads x 64), hp, col]
            qk = cpool.tile([128, 2, 2, HW], bf16)

            # p-state warmup on the junk tile (no DMA dependency): keeps PE
            # busy from ~0.7us so real projections dispatch at full clock
            for _ in range(NWARM):
                dummy = pproj.tile([128, 512], f32, tag="pp")
                nc.tensor.matmul(dummy, junk[:, 0:128], junk,
                                 start=True, stop=True)

            # eviction engines round-robin so no single finisher clogs
            _evict = {
                "dve": nc.vector.tensor_copy,
                "act": lambda o, i: nc.scalar.copy(o, i),
                "pool": nc.gpsimd.tensor_copy,
            }

            def emit_proj(c0, c1, hp, eng):
                """q+k for head-pair hp over cols [c0,c1) in one psum tile
                (<=256 cols -> one bank), single eviction on `eng`."""
                cw = c1 - c0
                pq = pproj.tile([128, 2, cw], f32, tag="pp")
                for qi in range(2):
                    blk = hp * 256 + qi * 128
                    for kt in range(KT):
                        nc.tensor.matmul(
                            pq[:, qi, :], wqk_sb[:, kt, blk:blk + 128],
                            hsT[:, kt, c0:c1],
                            start=(kt == 0), stop=(kt == KT - 1),
                        )
                _evict[eng](qk[:, :, hp, c0:c1], pq)

            tiles_E = {}

            def emit_head(t, split_exp=False):
                """Scores (transposed [j, i]) + exp; NEG band mask is the
                K=128 accumulation-group starter (a K=64 matmul must not
                open a PSUM group). split_exp halves the exp so the first
                head-pair's ctx can start earlier (head-phase tiles)."""
                mslot = 0 if t == 0 else (1 if t < NT - 1 else 2)
                moff = 128 + mslot * P
                icol = 7 + P * t
                jcol = P * t
                psc = pscore.tile([128, 464], f32, tag="psc")
                pscv = psc[:].rearrange("p (h m) -> p h m", h=4)[:, :, 0:P]
                for h in range(4):
                    hp = h // 2
                    pr = (h % 2) * 64
                    nc.tensor.matmul(
                        pscv[:, h, :], id_sb, cm[:, moff:moff + P],
                        start=True, stop=False,
                    )
                    nc.tensor.matmul(
                        pscv[:, h, :],
                        qk[pr:pr + 64, 1, hp, jcol:jcol + 128],
                        qk[pr:pr + 64, 0, hp, icol:icol + P],
                        start=False, stop=True,
                    )
                Em = epool.tile([128, 4, P], bf16, tag="Em")
                if split_exp:
                    nc.scalar.activation(Em[:, 0:2, :], pscv[:, 0:2, :], AF.Exp)
                    nc.scalar.activation(Em[:, 2:4, :], pscv[:, 2:4, :], AF.Exp)
                else:
                    nc.scalar.activation(Em, pscv[:, 0:4, :], AF.Exp)
                tiles_E[t] = (Em, psc)

            def _dve(o_, i):
                nc.vector.tensor_copy(o_, i)

            def _act(o_, i):
                nc.scalar.copy(o_, i)

            def emit_tail(t, split_out=False):
                """Transposed ctx: out^T[d, i] = win^T @ Em, one matmul per
                128-wide D-chunk with Em as the N=456 moving operand (4 heads
                at once). Four independent single-bank psum slots keep the
                evictions off the ring critical path. Band sums ride in
                chunk 0's spare psum cols [456:460) and leave as bf16 with
                the c0 eviction. Only DVE/Act may read PSUM; c3's eviction
                alternates by tile parity to balance them. split_out
                pipelines the out DMA with the evictions."""
                Em, psc = tiles_E.pop(t)
                Emf = Em[:].rearrange("p h i -> p (h i)")
                o = opool.tile([128, KT, OW], bf16, tag="o")
                ctx_evict = [_dve, _act, _dve, _act if t % 2 == 0 else _dve]
                for c in range(KT):
                    cw = OW if c == 0 else 4 * P
                    pc = pctx_pool.tile([128, cw], f32, tag=f"pctx{c}",
                                        bufs=1)
                    nc.tensor.matmul(pc[:, 0:4 * P], win[:, t, 128 * c:128 * (c + 1)],
                                     Emf, start=True, stop=True)
                    if c == 0:
                        for h in range(4):
                            nc.tensor.matmul(pc[0:P, 4 * P + h:4 * P + h + 1],
                                             Em[:, h, :], ones_sb,
                                             start=True, stop=True)
                    ctx_evict[c](o[:, c, 0:cw], pc)
                    if split_out and c % 2 == 1:
                        nc.sync.dma_start(out=out[:, c - 1:c + 1, t, :],
                                          in_=o[:, c - 1:c + 1, :])
                if not split_out:
                    nc.sync.dma_start(out=out[:, :, t, :], in_=o)

            emit_proj(0, 256, 0, "dve")
            emit_proj(0, 256, 1, "act")
            emit_head(0)
            emit_proj(256, 512, 0, "dve")
            emit_proj(256, 512, 1, "act")
            emit_head(1)
            emit_tail(0, split_out=True)
            emit_proj(512, 768, 0, "dve")
            emit_proj(512, 768, 1, "act")
            emit_head(2)
            emit_tail(1, split_out=True)
            emit_proj(768, 1024, 0, "act")
            emit_proj(768, 1024, 1, "act")
            emit_head(3)
            emit_tail(2, split_out=True)
            emit_proj(1024, HW, 0, "dve")
            emit_proj(1024, HW, 1, "act")
            for t in range(4, NT):
                emit_head(t)
                emit_tail(t - 1, split_out=True)
            emit_tail(NT - 1, split_out=True)

    nc.compile()
    return nc


def _get_nc():
    if "nc" not in _NC_CACHE:
        _NC_CACHE["nc"] = _build_nc()
    return _NC_CACHE["nc"]


def _band_mask(jmin, jmax):
    """[128, P] bf16 0/1 mask in [j, i] orientation."""
    j = np.arange(128)[:, None]
    i = np.arange(P)[None, :]
    valid = (j - i >= 0) & (j - i <= 14) & (j >= jmin) & (j <= jmax)
    return valid.astype(BF)


def kernel(hidden_states, Wq, Wk):
    global LAST_RESULTS
    from concourse import bass_utils

    B = hidden_states.shape[0]
    hs_bf = np.asarray(hidden_states).astype(BF)
    wq = np.asarray(Wq).astype(np.float32) * (1.0 / (DH ** 0.5))
    wk = np.asarray(Wk).astype(np.float32)

    in_maps = []
    for c in range(NCORES):
        b = c // 4
        sh = (c // 2) % 2
        hg = c % 2
        s0 = sh * SL

        pad = np.zeros((S + 16, D), BF)
        pad[7:7 + S] = hs_bf[b]

        hsd = np.ascontiguousarray(pad[s0:s0 + HW].T)            # [512, 1040]
        win = np.ascontiguousarray(
            np.stack([pad[s0 + P * t: s0 + P * t + 128] for t in range(NT)], axis=1)
        )                                                         # [128, 9, 512]

        wq_s = wq[:, hg * 256:(hg + 1) * 256]
        wk_s = wk[:, hg * 256:(hg + 1) * 256]
        packed = np.concatenate(
            [wq_s[:, 0:128], wk_s[:, 0:128], wq_s[:, 128:256], wk_s[:, 128:256]],
            axis=1,
        ).astype(BF)                                              # [512, 512]
        wqk = np.ascontiguousarray(packed.reshape(KT, 128, 512).transpose(1, 0, 2))

        # masks: slot0 (t=0), slot1 (interior), slot2 (t=8); j bounds clamp
        # the window to the batch (zero-padded rows must not survive)
        m0 = _band_mask(7 if sh == 0 else -1, 999)
        m1 = _band_mask(-1, 999)
        m2 = _band_mask(-1, 118 if sh == 1 else 999)
        neg = np.float32(-10000.0)
        m0, m1, m2 = (np.where(m > 0, 0.0, neg).astype(BF) for m in (m0, m1, m2))
        cmask = np.concatenate([np.eye(128, dtype=BF), m0, m1, m2,
                                np.ones((128, 2), BF)], axis=1)

        in_maps.append({"hsd": hsd, "win": win, "wqk": wqk, "cmask": cmask})

    nc = _get_nc()
    res = bass_utils.run_bass_kernel_spmd(
        nc, in_maps, core_ids=list(range(NCORES)), trace=TRACE,
    )
    LAST_RESULTS = res

    out = np.empty((B, H, S, D), np.float32)
    for c in range(NCORES):
        b = c // 4
        sh = (c // 2) % 2
        hg = c % 2
        s0 = sh * SL
        raw = np.asarray(res.results[c]["out"]).astype(np.float32)   # [128, 4, 9, 460]
        # out[p, ch, t, h*114+i] = ctx[d=128*ch+p, head h, row 114t+i];
        # [p<114, 0, t, 456+h] = band sum for (head h, row 114t+p)
        slab = (raw[:, :, :, 0:4 * P]
                   .transpose(1, 0, 2, 3)                            # [ch, p, t, 456]
                   .reshape(D, NT, 4, P)                             # [d, t, h, i]
                   .transpose(1, 3, 2, 0)                            # [t, i, h, d]
                   .reshape(NT * P, 4, D)[:SL])                      # [1024, 4, 512]
        s = raw[0:P, 0, :, 4 * P:]                                   # [114, 9, 4]
        s = s.transpose(1, 0, 2).reshape(NT * P, 4)[:SL]             # [1024, 4]
        slab /= (s + 1e-6)[:, :, None]
        out[b, 4 * hg:4 * hg + 4, s0:s0 + SL] = slab.transpose(1, 0, 2)
    return out
